# revision 9
# baseline (speedup 1.0000x reference)
"""GuidedFilterLayer Trainium2 kernel (8 NeuronCores, batch-sharded).

Math (derived from the reference):
    inputs   = (x+1)/2
    gray     = w0*R + w1*G + w2*B              (on x directly)
    guidance = 0.5*(gray + delta),  delta = mean(x) - mean(gray) + 1
    smoothed = box15(guidance)  (SAME zero pad) = (CB + delta*Wmap)/(225*2)
        where CB = colblur15(rowblur15(gray)) un-normalized, Wmap = wr (x) wc
        (in-bounds window counts)
    out      = 0.99*x - 0.01 + 0.02*smoothed
             = 0.99*x + [CB*(0.01/225) - 0.01] + (0.01*delta/225)*Wmap

Per core: 2 images, everything SBUF resident; one 1KB AllReduce for the
global channel sums (plus a warmup AllReduce at t=0 to pay the ncfw
first-collective setup concurrently with the load phase); row blur via
fp32 prefix scan; col blur via banded bf16 matmuls on TensorE.
"""

import numpy as np

B, H, W, C = 16, 512, 512, 3
NCORES = 8
B_LOC = B // NCORES          # 2 images per core
ROWS = B_LOC * H             # 1024 rows per core
FREE = W * C                 # 1536
NCHUNK = ROWS // 128         # 8 chunks of [128, 1536]
MPERIM = H // 128            # 4 row-chunks per image
NPIX = B * H * W             # global pixel count (for the means)
R_ = 7
K_ = 15
EPS = 0.01
W0, W1, W2 = 0.2989, 0.5870, 0.1140
# sum(x) = a1*acc1 + a2*acc2 + a3*acc3 from the gray-pass accumulators
# acc1=sum(w0*R), acc2=sum(w0*R+w1*G), acc3=sum(gray)
A1 = 1.0 / W0 - 1.0 / W1
A2 = 1.0 / W1 - 1.0 / W2
A3 = 1.0 / W2
SCALE_SM = EPS / (K_ * K_)    # 0.01/225
BIAS_SM = -EPS                # -0.01
CMAIN = 1.0 - EPS             # 0.99

_cache = {}


def _band_blocks():
    idx = np.arange(2 * 128)
    band = (np.abs(idx[:, None] - idx[None, :]) <= R_).astype(np.float32)
    bdiag = band[0:128, 0:128]        # kk == mm
    bup = band[0:128, 128:256]        # kk == mm-1  (rows above)
    bdn = band[128:256, 0:128]        # kk == mm+1  (rows below)
    return np.concatenate([bdiag, bup, bdn], axis=1)  # [128, 384]


def _wmap():
    i = np.arange(H)
    wr = (np.minimum(i + R_, H - 1) - np.maximum(i - R_, 0) + 1).astype(np.float32)
    return np.ascontiguousarray(wr[:, None] * wr[None, :])  # [512, 512]


def _build():
    from contextlib import ExitStack
    from concourse import bass, bacc, tile
    import concourse.mybir as mybir
    import ml_dtypes

    f32 = mybir.dt.float32
    bf16 = mybir.dt.bfloat16
    Alu = mybir.AluOpType
    Act = mybir.ActivationFunctionType

    nc = bacc.Bacc(
        "TRN2",
        target_bir_lowering=False,
        debug=False,
        enable_asserts=False,
        num_devices=NCORES,
    )

    x_in = nc.dram_tensor("x", [ROWS, FREE], f32, kind="ExternalInput")
    out_d = nc.dram_tensor("out", [ROWS, FREE], f32, kind="ExternalOutput")
    bands_d = nc.inline_tensor(
        _band_blocks().astype(ml_dtypes.bfloat16), name="bands")
    wmap_d = nc.inline_tensor(_wmap(), name="wmap")

    PADL = R_ + 1                  # 8 leading zeros in the scan buffer
    SW = PADL + W + R_             # 527

    with tile.TileContext(nc) as tc, ExitStack() as ctx:
        xp = ctx.enter_context(tc.tile_pool(name="xp", bufs=NCHUNK))
        gp = ctx.enter_context(tc.tile_pool(name="gp", bufs=2))
        sp = ctx.enter_context(tc.tile_pool(name="sp", bufs=2))
        rbp = ctx.enter_context(tc.tile_pool(name="rbp", bufs=NCHUNK))
        smp = ctx.enter_context(tc.tile_pool(name="smp", bufs=NCHUNK))
        sm2p = ctx.enter_context(tc.tile_pool(name="sm2p", bufs=3))
        op = ctx.enter_context(tc.tile_pool(name="op", bufs=3))
        cp = ctx.enter_context(tc.tile_pool(name="cp", bufs=1))
        pcb = ctx.enter_context(tc.tile_pool(name="pcb", bufs=2, space="PSUM"))
        dramp = ctx.enter_context(tc.tile_pool(name="dramp", bufs=1, space="DRAM"))

        # ---- warmup collective: pays ncfw setup + syncs core starts ----
        wu_sb = cp.tile([1, 128], f32, tag="wu_sb")
        nc.vector.memset(wu_sb[:], 0.0)
        wu_in = dramp.tile([1, 128], f32, tag="wu_in")
        wu_out = dramp.tile([1, 128], f32, tag="wu_out")
        nc.gpsimd.dma_start(out=wu_in[:], in_=wu_sb[:])
        nc.gpsimd.collective_compute(
            "AllReduce", mybir.AluOpType.add,
            replica_groups=[list(range(NCORES))],
            ins=[wu_in.opt()], outs=[wu_out.opt()])

        # constants to SBUF
        bsb = cp.tile([128, 384], bf16, tag="bands")
        nc.sync.dma_start(out=bsb[:], in_=bands_d[:])
        wm = []
        for m in range(MPERIM):
            t = cp.tile([128, W], f32, tag=f"wm{m}")
            nc.sync.dma_start(out=t[:], in_=wmap_d[128 * m:128 * (m + 1), :])
            wm.append(t)

        accs = cp.tile([128, 3 * NCHUNK], f32, tag="accs")  # acc1|acc2|acc3
        xts = []
        rbs = []
        sms = [None] * NCHUNK

        for t in range(NCHUNK):
            im, mm = divmod(t, MPERIM)
            xt = xp.tile([128, FREE], f32, tag="x")
            nc.sync.dma_start(out=xt[:], in_=x_in[128 * t:128 * (t + 1), :])
            xts.append(xt)
            x3 = xt[:].rearrange("p (w c) -> p c w", c=C)

            # gray = w0*R + w1*G + w2*B; first scaled copy on ScalarE
            ga = gp.tile([128, W], f32, tag="ga")
            gb = gp.tile([128, W], f32, tag="gb")
            gc = gp.tile([128, W], f32, tag="gc")
            nc.scalar.activation(
                out=ga[:], in_=x3[:, 0, :], func=Act.Copy, bias=0.0, scale=W0,
                accum_out=accs[:, t:t + 1])
            nc.vector.scalar_tensor_tensor(
                out=gb[:], in0=x3[:, 1, :], scalar=W1, in1=ga[:],
                op0=Alu.mult, op1=Alu.add,
                accum_out=accs[:, NCHUNK + t:NCHUNK + t + 1])
            nc.vector.scalar_tensor_tensor(
                out=gc[:], in0=x3[:, 2, :], scalar=W2, in1=gb[:],
                op0=Alu.mult, op1=Alu.add,
                accum_out=accs[:, 2 * NCHUNK + t:2 * NCHUNK + t + 1])

            # padded prefix scan: sbuf[0:8]=0, [8:520]=prefix(gc), [520:527]=S[511]
            st = sp.tile([128, SW], f32, tag="s")
            nc.vector.memset(st[:, 0:PADL], 0.0)
            nc.vector.tensor_tensor_scan(
                out=st[:, PADL:PADL + W], data0=gc[:], data1=gc[:], initial=0.0,
                op0=Alu.add, op1=Alu.bypass)
            nc.vector.tensor_copy(
                out=st[:, PADL + W:SW],
                in_=st[:, PADL + W - 1:PADL + W].broadcast_to([128, R_]))
            # rb[j] = S[j+7] - S[j-8]  (bf16 for the TensorE blur)
            rb = rbp.tile([128, W], bf16, tag="rb")
            nc.vector.tensor_tensor(
                out=rb[:], in0=st[:, K_:K_ + W], in1=st[:, 0:W], op=Alu.subtract)
            rbs.append(rb)

            if mm == MPERIM - 1:
                # image `im` complete: banded col-blur via TensorE
                for mo in range(MPERIM):
                    pc = pcb.tile([128, W], f32, tag="pc")
                    ks = [(mo, 0)]
                    if mo > 0:
                        ks.append((mo - 1, 1))
                    if mo < MPERIM - 1:
                        ks.append((mo + 1, 2))
                    for j, (kk, blk) in enumerate(ks):
                        nc.tensor.matmul(
                            out=pc[:],
                            lhsT=bsb[:, 128 * blk:128 * (blk + 1)],
                            rhs=rbs[im * MPERIM + kk][:],
                            start=(j == 0), stop=(j == len(ks) - 1))
                    sm = smp.tile([128, W], f32, tag="sm")
                    nc.scalar.activation(
                        out=sm[:], in_=pc[:], func=Act.Copy,
                        bias=BIAS_SM, scale=SCALE_SM)
                    sms[im * MPERIM + mo] = sm

        # ---- global sums -> AllReduce -> delta' ----
        red3 = cp.tile([128, 4], f32, tag="red3")
        for k in range(3):
            nc.vector.tensor_reduce(
                out=red3[:, k:k + 1], in_=accs[:, k * NCHUNK:(k + 1) * NCHUNK],
                axis=mybir.AxisListType.X, op=Alu.add)
        sb2 = cp.tile([128, 2], f32, tag="sb2")
        tmp = cp.tile([128, 2], f32, tag="tmp")
        # sum(x) rows = A1*r1 + A2*r2 + A3*r3 ; sum(gray) rows = r3
        nc.vector.tensor_scalar(
            out=tmp[:, 0:1], in0=red3[:, 0:1], scalar1=float(A1), scalar2=None,
            op0=Alu.mult)
        nc.vector.scalar_tensor_tensor(
            out=tmp[:, 1:2], in0=red3[:, 1:2], scalar=float(A2), in1=tmp[:, 0:1],
            op0=Alu.mult, op1=Alu.add)
        nc.vector.scalar_tensor_tensor(
            out=sb2[:, 0:1], in0=red3[:, 2:3], scalar=float(A3), in1=tmp[:, 1:2],
            op0=Alu.mult, op1=Alu.add)
        nc.vector.tensor_copy(out=sb2[:, 1:2], in_=red3[:, 2:3])

        cc_in = dramp.tile([128, 2], f32, tag="cc_in")
        cc_out = dramp.tile([128, 2], f32, tag="cc_out")
        nc.gpsimd.dma_start(out=cc_in[:], in_=sb2[:])
        nc.gpsimd.collective_compute(
            "AllReduce", mybir.AluOpType.add,
            replica_groups=[list(range(NCORES))],
            ins=[cc_in.opt()], outs=[cc_out.opt()])
        redg = cp.tile([128, 2], f32, tag="redg")
        nc.gpsimd.dma_start(out=redg[:], in_=cc_out[:])
        # cross-partition reduce + broadcast in one matmul with all-ones lhsT
        ones = cp.tile([128, 128], f32, tag="ones")
        nc.vector.memset(ones[:], 1.0)
        pred = pcb.tile([128, 2], f32, tag="pred")
        nc.tensor.matmul(out=pred[:], lhsT=ones[:], rhs=redg[:],
                         start=True, stop=True)
        redb = cp.tile([128, 2], f32, tag="redb")
        nc.scalar.copy(out=redb[:], in_=pred[:])

        # delta' = (sum(x)/(3N) - sum(gray)/N + 1) * (0.01/225), per partition
        d1 = cp.tile([128, 1], f32, tag="d1")
        d2 = cp.tile([128, 1], f32, tag="d2")
        d3 = cp.tile([128, 1], f32, tag="d3")
        nc.vector.tensor_scalar(
            out=d1[:], in0=redb[:, 0:1], scalar1=1.0 / (3.0 * NPIX), scalar2=None,
            op0=Alu.mult)
        nc.vector.scalar_tensor_tensor(
            out=d2[:], in0=redb[:, 1:2], scalar=-1.0 / NPIX, in1=d1[:],
            op0=Alu.mult, op1=Alu.add)
        nc.vector.tensor_scalar(
            out=d3[:], in0=d2[:], scalar1=1.0, scalar2=float(SCALE_SM),
            op0=Alu.add, op1=Alu.mult)

        # ---- final combine + store ----
        for t in range(NCHUNK):
            im, mm = divmod(t, MPERIM)
            sm2 = sm2p.tile([128, W], f32, tag="sm2")
            nc.vector.scalar_tensor_tensor(
                out=sm2[:], in0=wm[mm][:], scalar=d3[:], in1=sms[t][:],
                op0=Alu.mult, op1=Alu.add)
            ot = op.tile([128, FREE], f32, tag="o")
            o3 = ot[:].rearrange("p (w c) -> p w c", c=C)
            x3f = xts[t][:].rearrange("p (w c) -> p w c", c=C)
            nc.vector.scalar_tensor_tensor(
                out=o3, in0=x3f, scalar=float(CMAIN),
                in1=sm2[:].broadcast_to([128, W, C]),
                op0=Alu.mult, op1=Alu.add)
            nc.sync.dma_start(out=out_d[128 * t:128 * (t + 1), :], in_=ot[:])

    nc.finalize()
    return nc


def _get_nc():
    if "nc" not in _cache:
        _cache["nc"] = _build()
    return _cache["nc"]


def kernel(x):
    from concourse.bass_utils import run_bass_kernel_spmd

    x = np.ascontiguousarray(np.asarray(x, dtype=np.float32))
    assert x.shape == (B, H, W, C)
    nc = _get_nc()
    in_maps = [
        {"x": np.ascontiguousarray(
            x[i * B_LOC:(i + 1) * B_LOC].reshape(ROWS, FREE))}
        for i in range(NCORES)
    ]
    res = run_bass_kernel_spmd(nc, in_maps, core_ids=list(range(NCORES)))
    out = np.concatenate(
        [res.results[i]["out"].reshape(B_LOC, H, W, C) for i in range(NCORES)],
        axis=0,
    )
    return out
